# revision 11
# baseline (speedup 1.0000x reference)
"""Trainium2 Bass kernel for unscaled cross-attention (key doubles as value).

Problem: B=8, Tq=Tk=2048, D=1024, fp32.
  energy = Q @ K^T  ->  softmax over Tk  ->  out = attn @ K

Sharding: batch dim across the 8 NeuronCores (1 batch element per core).

Per-core algorithm (all matmul operands in float16: full PE rate like
f32r/bf16, but 16-bit weight loads get FWL so LDWEIGHTS never throttles
the PE, and 10-bit mantissa keeps rel err ~2e-3):
  prologue: kick all K DMAs (1MB chunks); as each chunk lands, cast to f16
            (knat, resident, MM2 rhs), PE-transpose to K^T [d,k] f16
            (resident, MM1 rhs), and run q-block 0's MM1 over the freshly
            available 256 columns -- the PE never waits for more than one
            chunk and stays warm (HAM) through the K stream-in.
  software-pipelined main loop over 16 q-blocks (128 rows each):
    stage A(i):  load Q block i, cast f16, PE-transpose -> qt [d,q]
    stage B(i):  S = qt.T @ K^T (PSUM [128,2048] f32, 512-col chunks, 8
                 d-tile accumulation per chunk; per-chunk rowmax on DVE)
                 negmax -> P = exp(S+negmax) per chunk, f16 out, fused
                 rowsum (ACT); recip = 1/sum (DVE)
    stage C(i):  P^T via f16 PE transposes (two 8-wide waves; DVE drain of
                 wave 1 hides under wave 2); O = P^T.T @ K_nat as two
                 512-col accumulation groups, each scaled (ACT) and stored
                 as soon as it completes (shrinks the kernel tail).
  Emission order per iteration: A(i+1), B(i+1) matmuls, C(i) -- stage C's
  PE work fills the wait for block i+1's softmax chain on DVE/ACT.
"""

import sys

if "/opt/trn_rl_repo" not in sys.path:
    sys.path.insert(0, "/opt/trn_rl_repo")

import numpy as np

import concourse.bacc as bacc
import concourse.tile as tile
from concourse import mybir
from concourse.bass_utils import run_bass_kernel_spmd
from concourse.masks import make_identity

N_CORES = 8
T = 2048          # Tq == Tk
D = 1024
P = 128
DO = D // P       # 8 d-tiles
KO = T // P       # 16 k-tiles
QB = T // P       # 16 q-blocks
NC4 = T // 512    # 4 S chunks
NC8 = T // 256    # 8 prologue half-chunks
F32 = mybir.dt.float32
F16 = mybir.dt.float16


def build_body(nc, tc, ctx, q_ap, k_ap, out_ap, n_reps=1):
    const = ctx.enter_context(tc.tile_pool(name="const", bufs=1))
    kt_pool = ctx.enter_context(tc.tile_pool(name="kt", bufs=1))
    knat_pool = ctx.enter_context(tc.tile_pool(name="knat", bufs=1))
    ld_pool = ctx.enter_context(tc.tile_pool(name="ld", bufs=4))
    qr_pool = ctx.enter_context(tc.tile_pool(name="qr", bufs=2))
    qt_pool = ctx.enter_context(tc.tile_pool(name="qt", bufs=2))
    p_pool = ctx.enter_context(tc.tile_pool(name="p", bufs=2))
    pt_pool = ctx.enter_context(tc.tile_pool(name="pt", bufs=2))
    o_pool = ctx.enter_context(tc.tile_pool(name="o", bufs=2))
    stat_pool = ctx.enter_context(tc.tile_pool(name="stat", bufs=8))
    s_psum = ctx.enter_context(tc.tile_pool(name="s_ps", bufs=1, space="PSUM"))
    tr_psum = ctx.enter_context(tc.tile_pool(name="tr_ps", bufs=2, space="PSUM"))
    o_psum = ctx.enter_context(tc.tile_pool(name="o_ps", bufs=2, space="PSUM"))

    ident = const.tile([P, P], F32)
    make_identity(nc, ident)
    ident_h = const.tile([P, P], F16)
    nc.vector.tensor_copy(out=ident_h, in_=ident)

    kt_c = [kt_pool.tile([P, DO, 512], F16, name=f"ktc{c}", tag=f"ktc{c}")
            for c in range(NC4)]              # kt_c[c][dd, do, kk] = K[c*512+kk, do*128+dd]
    knat = knat_pool.tile([P, KO, D], F16)    # knat[kk, ko, d] = K[ko*128+kk, d]

    # ---- prologue helpers: 1MB K chunks (2 row-blocks) -> knat + kt ----
    def k_dma(ko2):
        kc = ld_pool.tile([P, 2, D], F32, tag="ldk", name="kc")
        nc.sync.dma_start(
            out=kc,
            in_=k_ap[ko2 * 2 * P:(ko2 + 1) * 2 * P, :].rearrange(
                "(t p) d -> p t d", p=P),
        )
        return kc

    def k_process(ko2, kc):
        nc.vector.tensor_copy(out=knat[:, ko2 * 2:(ko2 + 1) * 2, :], in_=kc)
        for ko in (ko2 * 2, ko2 * 2 + 1):
            trt = tr_psum.tile([P, DO * P], F16, tag="tr", name="trt")
            for do in range(DO):
                nc.tensor.transpose(
                    trt[:, do * P:(do + 1) * P],
                    knat[:, ko, do * P:(do + 1) * P], ident_h
                )
            nc.vector.tensor_copy(
                out=kt_c[ko // 4][:, :, (ko % 4) * P:(ko % 4 + 1) * P],
                in_=trt.rearrange("p (do f) -> p do f", do=DO),
            )

    # ---- software-pipelined main loop ----
    def stage_a(qb):
        """DMA + cast f16 + PE-transpose one Q block -> qt [d, q] tiles."""
        qc = ld_pool.tile([P, D], F32, tag="ld", name="qc")
        nc.sync.dma_start(out=qc, in_=q_ap[qb * P:(qb + 1) * P, :])
        qr = qr_pool.tile([P, D], F16, tag="qr", name="qr")
        nc.vector.tensor_copy(out=qr, in_=qc)          # cast f32 -> f16
        qt = qt_pool.tile([P, DO, P], F16, tag="qt", name="qt")
        trt = tr_psum.tile([P, DO * P], F16, tag="tr", name="trt")
        for do in range(DO):
            nc.tensor.transpose(
                trt[:, do * P:(do + 1) * P], qr[:, do * P:(do + 1) * P],
                ident_h
            )
        nc.vector.tensor_copy(
            out=qt, in_=trt.rearrange("p (do f) -> p do f", do=DO),
        )
        return qt

    def stage_b_open(nseg):
        s_ps = s_psum.tile([P, T], F32, tag="s", name="s_ps")
        seg_max = stat_pool.tile([P, nseg], F32, tag="segmax", name="seg_max")
        return s_ps, seg_max

    def stage_b_chunk(s_ps, seg_max, qt, c4):
        for do in range(DO):
            nc.tensor.matmul(
                s_ps[:, c4 * 512:(c4 + 1) * 512],
                lhsT=qt[:, do, :],
                rhs=kt_c[c4][:, do, :],
                start=(do == 0),
                stop=(do == DO - 1),
            )
        nc.vector.tensor_reduce(
            out=seg_max[:, c4:c4 + 1], in_=s_ps[:, c4 * 512:(c4 + 1) * 512],
            axis=mybir.AxisListType.X, op=mybir.AluOpType.max,
        )

    def stage_b_halfchunk(s_ps, seg_max, qt, h8):
        """256-col MM1 slice: needs only K blocks 2*h8, 2*h8+1 (one DMA)."""
        c4, half = h8 // 2, h8 % 2
        for do in range(DO):
            nc.tensor.matmul(
                s_ps[:, h8 * 256:(h8 + 1) * 256],
                lhsT=qt[:, do, :],
                rhs=kt_c[c4][:, do, half * 256:(half + 1) * 256],
                start=(do == 0),
                stop=(do == DO - 1),
            )
        nc.vector.tensor_reduce(
            out=seg_max[:, h8:h8 + 1], in_=s_ps[:, h8 * 256:(h8 + 1) * 256],
            axis=mybir.AxisListType.X, op=mybir.AluOpType.max,
        )

    def stage_b_exp(s_ps, seg_max):
        """negmax + chunked exp (f16 out) with fused row-sums."""
        negmax = stat_pool.tile([P, 1], F32, tag="negmax", name="negmax")
        nc.vector.tensor_reduce(
            out=negmax, in_=seg_max, axis=mybir.AxisListType.X,
            op=mybir.AluOpType.max, negate=True,
        )
        p_sb = p_pool.tile([P, T], F16, tag="p", name="p_sb")
        sum4 = stat_pool.tile([P, NC4], F32, tag="sum4", name="sum4")
        for c4 in range(NC4):
            nc.scalar.activation(
                out=p_sb[:, c4 * 512:(c4 + 1) * 512],
                in_=s_ps[:, c4 * 512:(c4 + 1) * 512],
                func=mybir.ActivationFunctionType.Exp,
                bias=negmax, scale=1.0,
                accum_out=sum4[:, c4:c4 + 1],
            )
        return p_sb, sum4

    def stage_b_finish(sum4):
        sumexp = stat_pool.tile([P, 1], F32, tag="sumexp", name="sumexp")
        nc.vector.tensor_reduce(
            out=sumexp, in_=sum4, axis=mybir.AxisListType.X,
            op=mybir.AluOpType.add,
        )
        recip = stat_pool.tile([P, 1], F32, tag="recip", name="recip")
        nc.vector.reciprocal(recip, sumexp)
        return recip

    def stage_c(qb, p_sb, recip):
        """P^T f16 PE transposes + MM2 (f16) + scale + store for one block.

        Two 8-wide transpose waves (one PSUM bank each); the DVE drain of
        wave 1 hides under the PE time of wave 2.  MM2 runs as two
        independent 512-col accumulation groups so the first half's
        scale+store overlaps the second half's matmuls.
        """
        pt = pt_pool.tile([P, KO, P], F16, tag="pt", name="pt")

        def tr_wave(w):
            trt = tr_psum.tile([P, DO * P], F16, tag="tr", name="trt")
            for j in range(8):
                ko = w * 8 + j
                nc.tensor.transpose(
                    trt[:, j * P:(j + 1) * P], p_sb[:, ko * P:(ko + 1) * P],
                    ident_h
                )
            nc.vector.tensor_copy(
                out=pt[:, w * 8:(w + 1) * 8, :],
                in_=trt.rearrange("p (j f) -> p j f", j=8),
            )

        def mm2_half(o_ps, c):
            for ko in range(KO):
                nc.tensor.matmul(
                    o_ps[:, :],
                    lhsT=pt[:, ko, :],
                    rhs=knat[:, ko, c * 512:(c + 1) * 512],
                    start=(ko == 0),
                    stop=(ko == KO - 1),
                )

        def scale_store(o_ps, c):
            o_sb = o_pool.tile([P, 512], F32, tag="o_sb", name="o_sb")
            nc.scalar.activation(
                out=o_sb, in_=o_ps, func=mybir.ActivationFunctionType.Copy,
                scale=recip,
            )
            nc.sync.dma_start(
                out=out_ap[qb * P:(qb + 1) * P, c * 512:(c + 1) * 512],
                in_=o_sb)

        tr_wave(0)
        tr_wave(1)
        o_ps0 = o_psum.tile([P, 512], F32, tag="o", name="o_ps0")
        mm2_half(o_ps0, 0)
        scale_store(o_ps0, 0)
        o_ps1 = o_psum.tile([P, 512], F32, tag="o", name="o_ps1")
        mm2_half(o_ps1, 1)
        scale_store(o_ps1, 1)

    for rep in range(n_reps):
        qt = stage_a(0)
        if rep == 0:
            # kick all K loads upfront (ld bufs bound the in-flight count),
            # then per landed 1MB chunk: cast+transpose it and run block 0's
            # MM1 over the freshly available 256 columns.
            s_ps, seg_max = stage_b_open(NC8)
            kcs = [k_dma(ko2) for ko2 in range(2 * NC4)]
            # halfchunk h8 lags k_process(h8) by one chunk so the DVE drain
            # of kt chunk h8 hides under the PE transposes of chunk h8+1
            for h8 in range(NC8):
                k_process(h8, kcs[h8])
                if h8 >= 1:
                    stage_b_halfchunk(s_ps, seg_max, qt, h8 - 1)
            stage_b_halfchunk(s_ps, seg_max, qt, NC8 - 1)
        else:
            s_ps, seg_max = stage_b_open(NC4)
            for c4 in range(NC4):
                stage_b_chunk(s_ps, seg_max, qt, c4)
        p_sb, sum4 = stage_b_exp(s_ps, seg_max)
        qt = stage_a(1)
        prev_p, prev_recip = p_sb, stage_b_finish(sum4)
        for qb in range(1, QB):
            s_ps, seg_max = stage_b_open(NC4)
            for c4 in range(NC4):
                stage_b_chunk(s_ps, seg_max, qt, c4)
            p_sb, sum4 = stage_b_exp(s_ps, seg_max)
            if qb + 1 < QB:
                qt = stage_a(qb + 1)
            stage_c(qb - 1, prev_p, prev_recip)
            prev_p, prev_recip = p_sb, stage_b_finish(sum4)
        stage_c(QB - 1, prev_p, prev_recip)


def build_nc(n_reps=1):
    from contextlib import ExitStack

    nc = bacc.Bacc("TRN2", target_bir_lowering=False, debug=False,
                   num_devices=N_CORES)
    q_ap = nc.dram_tensor("q", [T, D], F32, kind="ExternalInput").ap()
    k_ap = nc.dram_tensor("k", [T, D], F32, kind="ExternalInput").ap()
    out_ap = nc.dram_tensor("out", [T, D], F32, kind="ExternalOutput").ap()
    with tile.TileContext(nc) as tc:
        with ExitStack() as ctx:
            build_body(nc, tc, ctx, q_ap, k_ap, out_ap, n_reps=n_reps)
    nc.compile()
    return nc


_nc_cache = {}


def kernel(query: np.ndarray, key: np.ndarray) -> np.ndarray:
    """Full unsharded inputs [8, 2048, 1024] fp32 -> output [8, 2048, 1024]."""
    assert query.shape == (N_CORES, T, D) and key.shape == (N_CORES, T, D)
    if "nc" not in _nc_cache:
        _nc_cache["nc"] = build_nc()
    nc = _nc_cache["nc"]
    in_maps = [
        {"q": np.ascontiguousarray(query[b], dtype=np.float32),
         "k": np.ascontiguousarray(key[b], dtype=np.float32)}
        for b in range(N_CORES)
    ]
    res = run_bass_kernel_spmd(nc, in_maps, list(range(N_CORES)))
    out = np.stack([res.results[b]["out"] for b in range(N_CORES)], axis=0)
    return out.astype(np.float32)


# revision 13
# speedup vs baseline: 1.2034x; 1.2034x over previous
"""Trainium2 Bass kernel for unscaled cross-attention (key doubles as value).

Problem: B=8, Tq=Tk=2048, D=1024, fp32.
  energy = Q @ K^T  ->  softmax over Tk  ->  out = attn @ K

Sharding: batch dim across the 8 NeuronCores (1 batch element per core).

Per-core algorithm (all matmul operands in float16: full PE rate like
f32r/bf16, but 16-bit weight loads get FWL so LDWEIGHTS never throttles
the PE, and 10-bit mantissa keeps rel err ~2e-3):
  prologue: kick all K DMAs (1MB chunks); as each chunk lands, cast to f16
            (knat, resident, MM2 rhs), PE-transpose to K^T [d,k] f16
            (resident, MM1 rhs), and run q-block 0's MM1 over the freshly
            available 256 columns -- the PE never waits for more than one
            chunk and stays warm (HAM) through the K stream-in.
  software-pipelined main loop over 16 q-blocks (128 rows each):
    stage A(i):  load Q block i, cast f16, PE-transpose -> qt [d,q]
    stage B(i):  S = qt.T @ K^T (PSUM [128,2048] f32, 512-col chunks, 8
                 d-tile accumulation per chunk; per-chunk rowmax on DVE)
                 negmax -> P = exp(S+negmax) per chunk, f16 out, fused
                 rowsum (ACT); recip = 1/sum (DVE)
    stage C(i):  P^T via f16 PE transposes (two 8-wide waves; DVE drain of
                 wave 1 hides under wave 2); O = P^T.T @ K_nat as two
                 512-col accumulation groups, each scaled (ACT) and stored
                 as soon as it completes (shrinks the kernel tail).
  Emission order per iteration: A(i+1), B(i+1) matmuls, C(i) -- stage C's
  PE work fills the wait for block i+1's softmax chain on DVE/ACT.
"""

import sys

if "/opt/trn_rl_repo" not in sys.path:
    sys.path.insert(0, "/opt/trn_rl_repo")

import numpy as np

import concourse.bacc as bacc
import concourse.tile as tile
from concourse import mybir
from concourse.bass_utils import run_bass_kernel_spmd
from concourse.masks import make_identity

N_CORES = 8
T = 2048          # Tq == Tk
D = 1024
P = 128
DO = D // P       # 8 d-tiles
KO = T // P       # 16 k-tiles
QB = T // P       # 16 q-blocks
NC4 = T // 512    # 4 S chunks
NC8 = T // 256    # 8 prologue half-chunks
F32 = mybir.dt.float32
F16 = mybir.dt.float16


def build_body(nc, tc, ctx, q_ap, k_ap, out_ap, n_reps=1):
    const = ctx.enter_context(tc.tile_pool(name="const", bufs=1))
    kt_pool = ctx.enter_context(tc.tile_pool(name="kt", bufs=1))
    knat_pool = ctx.enter_context(tc.tile_pool(name="knat", bufs=1))
    ld_pool = ctx.enter_context(tc.tile_pool(name="ld", bufs=4))
    qr_pool = ctx.enter_context(tc.tile_pool(name="qr", bufs=2))
    qt_pool = ctx.enter_context(tc.tile_pool(name="qt", bufs=2))
    p_pool = ctx.enter_context(tc.tile_pool(name="p", bufs=2))
    pt_pool = ctx.enter_context(tc.tile_pool(name="pt", bufs=2))
    o_pool = ctx.enter_context(tc.tile_pool(name="o", bufs=2))
    stat_pool = ctx.enter_context(tc.tile_pool(name="stat", bufs=8))
    s_psum = ctx.enter_context(tc.tile_pool(name="s_ps", bufs=1, space="PSUM"))
    tr_psum = ctx.enter_context(tc.tile_pool(name="tr_ps", bufs=2, space="PSUM"))
    o_psum = ctx.enter_context(tc.tile_pool(name="o_ps", bufs=2, space="PSUM"))

    ident = const.tile([P, P], F32)
    make_identity(nc, ident)
    ident_h = const.tile([P, P], F16)
    nc.vector.tensor_copy(out=ident_h, in_=ident)

    kt_c = [kt_pool.tile([P, DO, 512], F16, name=f"ktc{c}", tag=f"ktc{c}")
            for c in range(NC4)]              # kt_c[c][dd, do, kk] = K[c*512+kk, do*128+dd]
    knat = knat_pool.tile([P, KO, D], F16)    # knat[kk, ko, d] = K[ko*128+kk, d]

    # ---- prologue helpers: 1MB K chunks (2 row-blocks) -> knat + kt ----
    def k_dma(ko2):
        kc = ld_pool.tile([P, 2, D], F32, tag="ldk", name="kc")
        nc.sync.dma_start(
            out=kc,
            in_=k_ap[ko2 * 2 * P:(ko2 + 1) * 2 * P, :].rearrange(
                "(t p) d -> p t d", p=P),
        )
        return kc

    def k_process(ko2, kc):
        # cast on ACT (idle in the prologue) -- the DVE is busy with kt
        # drains + rowmax reduces and would delay the PE otherwise
        nc.scalar.activation(
            out=knat[:, ko2 * 2:(ko2 + 1) * 2, :], in_=kc,
            func=mybir.ActivationFunctionType.Copy, scale=1.0,
        )
        for ko in (ko2 * 2, ko2 * 2 + 1):
            trt = tr_psum.tile([P, DO * P], F16, tag="tr", name="trt")
            for do in range(DO):
                nc.tensor.transpose(
                    trt[:, do * P:(do + 1) * P],
                    knat[:, ko, do * P:(do + 1) * P], ident_h
                )
            nc.vector.tensor_copy(
                out=kt_c[ko // 4][:, :, (ko % 4) * P:(ko % 4 + 1) * P],
                in_=trt.rearrange("p (do f) -> p do f", do=DO),
            )

    # ---- software-pipelined main loop ----
    def stage_a(qb):
        """DMA + cast f16 + PE-transpose one Q block -> qt [d, q] tiles."""
        qc = ld_pool.tile([P, D], F32, tag="ld", name="qc")
        nc.sync.dma_start(out=qc, in_=q_ap[qb * P:(qb + 1) * P, :])
        qr = qr_pool.tile([P, D], F16, tag="qr", name="qr")
        nc.scalar.activation(                          # cast f32 -> f16 (ACT)
            out=qr, in_=qc,
            func=mybir.ActivationFunctionType.Copy, scale=1.0,
        )
        qt = qt_pool.tile([P, DO, P], F16, tag="qt", name="qt")
        trt = tr_psum.tile([P, DO * P], F16, tag="tr", name="trt")
        for do in range(DO):
            nc.tensor.transpose(
                trt[:, do * P:(do + 1) * P], qr[:, do * P:(do + 1) * P],
                ident_h
            )
        nc.vector.tensor_copy(
            out=qt, in_=trt.rearrange("p (do f) -> p do f", do=DO),
        )
        return qt

    def stage_b_open(nseg):
        s_ps = s_psum.tile([P, T], F32, tag="s", name="s_ps")
        seg_max = stat_pool.tile([P, nseg], F32, tag="segmax", name="seg_max")
        return s_ps, seg_max

    def stage_b_chunk(s_ps, seg_max, qt, c4):
        for do in range(DO):
            nc.tensor.matmul(
                s_ps[:, c4 * 512:(c4 + 1) * 512],
                lhsT=qt[:, do, :],
                rhs=kt_c[c4][:, do, :],
                start=(do == 0),
                stop=(do == DO - 1),
            )
        nc.vector.tensor_reduce(
            out=seg_max[:, c4:c4 + 1], in_=s_ps[:, c4 * 512:(c4 + 1) * 512],
            axis=mybir.AxisListType.X, op=mybir.AluOpType.max,
        )

    def stage_b_halfchunk(s_ps, seg_max, qt, h8):
        """256-col MM1 slice: needs only K blocks 2*h8, 2*h8+1 (one DMA)."""
        c4, half = h8 // 2, h8 % 2
        for do in range(DO):
            nc.tensor.matmul(
                s_ps[:, h8 * 256:(h8 + 1) * 256],
                lhsT=qt[:, do, :],
                rhs=kt_c[c4][:, do, half * 256:(half + 1) * 256],
                start=(do == 0),
                stop=(do == DO - 1),
            )
        nc.vector.tensor_reduce(
            out=seg_max[:, h8:h8 + 1], in_=s_ps[:, h8 * 256:(h8 + 1) * 256],
            axis=mybir.AxisListType.X, op=mybir.AluOpType.max,
        )

    def stage_b_exp(s_ps, seg_max):
        """negmax + chunked exp (f16 out) with fused row-sums."""
        negmax = stat_pool.tile([P, 1], F32, tag="negmax", name="negmax")
        nc.vector.tensor_reduce(
            out=negmax, in_=seg_max, axis=mybir.AxisListType.X,
            op=mybir.AluOpType.max, negate=True,
        )
        p_sb = p_pool.tile([P, T], F16, tag="p", name="p_sb")
        sum4 = stat_pool.tile([P, NC4], F32, tag="sum4", name="sum4")
        for c4 in range(NC4):
            nc.scalar.activation(
                out=p_sb[:, c4 * 512:(c4 + 1) * 512],
                in_=s_ps[:, c4 * 512:(c4 + 1) * 512],
                func=mybir.ActivationFunctionType.Exp,
                bias=negmax, scale=1.0,
                accum_out=sum4[:, c4:c4 + 1],
            )
        return p_sb, sum4

    def stage_b_finish(sum4):
        sumexp = stat_pool.tile([P, 1], F32, tag="sumexp", name="sumexp")
        nc.vector.tensor_reduce(
            out=sumexp, in_=sum4, axis=mybir.AxisListType.X,
            op=mybir.AluOpType.add,
        )
        recip = stat_pool.tile([P, 1], F32, tag="recip", name="recip")
        nc.vector.reciprocal(recip, sumexp)
        return recip

    def stage_c(qb, p_sb, recip):
        """P^T f16 PE transposes + MM2 (f16) + scale + store for one block.

        Two 8-wide transpose waves (one PSUM bank each); the DVE drain of
        wave 1 hides under the PE time of wave 2.  MM2 runs as two
        independent 512-col accumulation groups so the first half's
        scale+store overlaps the second half's matmuls.
        """
        pt = pt_pool.tile([P, KO, P], F16, tag="pt", name="pt")

        def tr_wave(w):
            trt = tr_psum.tile([P, DO * P], F16, tag="tr", name="trt")
            for j in range(8):
                ko = w * 8 + j
                nc.tensor.transpose(
                    trt[:, j * P:(j + 1) * P], p_sb[:, ko * P:(ko + 1) * P],
                    ident_h
                )
            nc.vector.tensor_copy(
                out=pt[:, w * 8:(w + 1) * 8, :],
                in_=trt.rearrange("p (j f) -> p j f", j=8),
            )

        def mm2_half(o_ps, c):
            for ko in range(KO):
                nc.tensor.matmul(
                    o_ps[:, :],
                    lhsT=pt[:, ko, :],
                    rhs=knat[:, ko, c * 512:(c + 1) * 512],
                    start=(ko == 0),
                    stop=(ko == KO - 1),
                )

        def scale_store(o_ps, c):
            o_sb = o_pool.tile([P, 512], F32, tag="o_sb", name="o_sb")
            nc.scalar.activation(
                out=o_sb, in_=o_ps, func=mybir.ActivationFunctionType.Copy,
                scale=recip,
            )
            nc.sync.dma_start(
                out=out_ap[qb * P:(qb + 1) * P, c * 512:(c + 1) * 512],
                in_=o_sb)

        tr_wave(0)
        tr_wave(1)
        o_ps0 = o_psum.tile([P, 512], F32, tag="o", name="o_ps0")
        mm2_half(o_ps0, 0)
        scale_store(o_ps0, 0)
        o_ps1 = o_psum.tile([P, 512], F32, tag="o", name="o_ps1")
        mm2_half(o_ps1, 1)
        scale_store(o_ps1, 1)

    for rep in range(n_reps):
        qt = stage_a(0)
        if rep == 0:
            # kick all K loads upfront (ld bufs bound the in-flight count),
            # then per landed 1MB chunk: cast+transpose it and run block 0's
            # MM1 over the freshly available 256 columns.
            s_ps, seg_max = stage_b_open(NC8)
            kcs = [k_dma(ko2) for ko2 in range(2 * NC4)]
            # halfchunk h8 lags k_process(h8) by one chunk so the DVE drain
            # of kt chunk h8 hides under the PE transposes of chunk h8+1
            for h8 in range(NC8):
                k_process(h8, kcs[h8])
                if h8 >= 1:
                    stage_b_halfchunk(s_ps, seg_max, qt, h8 - 1)
            stage_b_halfchunk(s_ps, seg_max, qt, NC8 - 1)
        else:
            s_ps, seg_max = stage_b_open(NC4)
            for c4 in range(NC4):
                stage_b_chunk(s_ps, seg_max, qt, c4)
        p_sb, sum4 = stage_b_exp(s_ps, seg_max)
        qt = stage_a(1)
        prev_p, prev_recip = p_sb, stage_b_finish(sum4)
        for qb in range(1, QB):
            s_ps, seg_max = stage_b_open(NC4)
            for c4 in range(NC4):
                stage_b_chunk(s_ps, seg_max, qt, c4)
            p_sb, sum4 = stage_b_exp(s_ps, seg_max)
            if qb + 1 < QB:
                qt = stage_a(qb + 1)
            stage_c(qb - 1, prev_p, prev_recip)
            prev_p, prev_recip = p_sb, stage_b_finish(sum4)
        stage_c(QB - 1, prev_p, prev_recip)


def build_nc(n_reps=1):
    from contextlib import ExitStack

    nc = bacc.Bacc("TRN2", target_bir_lowering=False, debug=False,
                   num_devices=N_CORES)
    q_ap = nc.dram_tensor("q", [T, D], F32, kind="ExternalInput").ap()
    k_ap = nc.dram_tensor("k", [T, D], F32, kind="ExternalInput").ap()
    out_ap = nc.dram_tensor("out", [T, D], F32, kind="ExternalOutput").ap()
    with tile.TileContext(nc) as tc:
        with ExitStack() as ctx:
            build_body(nc, tc, ctx, q_ap, k_ap, out_ap, n_reps=n_reps)
    nc.compile()
    return nc


_nc_cache = {}


def kernel(query: np.ndarray, key: np.ndarray) -> np.ndarray:
    """Full unsharded inputs [8, 2048, 1024] fp32 -> output [8, 2048, 1024]."""
    assert query.shape == (N_CORES, T, D) and key.shape == (N_CORES, T, D)
    if "nc" not in _nc_cache:
        _nc_cache["nc"] = build_nc()
    nc = _nc_cache["nc"]
    in_maps = [
        {"q": np.ascontiguousarray(query[b], dtype=np.float32),
         "k": np.ascontiguousarray(key[b], dtype=np.float32)}
        for b in range(N_CORES)
    ]
    res = run_bass_kernel_spmd(nc, in_maps, list(range(N_CORES)))
    out = np.stack([res.results[b]["out"] for b in range(N_CORES)], axis=0)
    return out.astype(np.float32)
